# revision 60
# baseline (speedup 1.0000x reference)
"""Trainium2 Bass kernel for nn_AttentionEnhancedBiLSTM (8 NeuronCores, SPMD).

Math (from the reference):
    x  = inputs[:, -1, :]                        # [B=1024, E=1024]
    af = softmax((x Wq^T)(x Wk^T)^T / 32) (x Wv^T) Wo^T + bo     (fwd dir)
    h_f = sigmoid(o) * tanh(sigmoid(i) * tanh(g)),  gates = (af+x) W_ih^T + b
    backward: same with xr = x[:, ::-1] and its own weights; output keeps the
    CELL state c_b = sigmoid(i)*tanh(g).
    out = concat([h_f, c_b], -1)                 # [1024, 1024]

Weight-only host folds (input-independent preprocessing, exact):
    A    = (Wq^T Wk) / 32            -> scores = x A x^T   (kills Wk + 1 stage)
    Wvo  = Wo Wv                     -> av+outproj = (p @ x) @ Wvo^T (kills Wo)
    backward column-flips of x are folded into A_b / WvoT_b, so the device
    only ever sees the unflipped x (no xr tensors, no flipped DMAs).
    biases: the k/q score biases are softmax-row-invariant except a rank-1
    term folded as a bias row of t; v/o biases ride through softmax (rows of
    p sum to 1) and fold into the LSTM bias via bih += W_sel (Wo bv + bo).

Sharding: batch-sharded 8 ways (128 rows/core), ZERO collectives. Each core
reads the folded weights + x^T (full, 2MB) in bf16: ~15.5MB HBM per core vs
46MB for the unfolded TP/allgather version. The x-natural layout needed by
the u matmul is derived on-chip with PE transposes instead of a second 2MB
DMA (HBM bytes are the binding resource; sim: DMA 46.6us vs PE 43us busy).

Device pipeline per direction (matmul operands bf16, PSUM f32):
    tT[e',i]   = sum_e  A[e,e'] xT[e,i]       (64 mm, fused proj+transpose)
    scores[i,j]= sum_e' tT[e',i] xT[e',j]     (16 mm, N=512)
    p_norm     = exp(scores)/rowsum  (no max-subtract: scores ~ N(0,1); the
                 rowsum comes free via the Act accumulator)
    pT         = transpose(p_norm)            (8 PE transposes)
    uT[e,i]    = sum_j  x[j,e] pT[j,i]        (64 mm: u = p @ x, transposed)
    avT[e~,i]  = sum_e  WvoT[e,e~] uT[e,i]    (64 mm)
    lstm_T     = avT + xTs                    (DVE add, stays transposed)
    gates      = lstm_T^T @ WihT + bih        (24/16 mm + ones-matmul bias)
    h/c        = LSTM cell nonlinearities     (Act + DVE, b-dir tail
                 pipelined in halves straight into the output DMA)

Scheduling notes (all validated against the TimelineSim cost model):
  - matmul start=True zeroes the whole 2KB PSUM bank, so in the 128-wide
    fused matmuls only the first block per bank asserts it.
  - one in-order DMA queue (SP) carries every weight/x stream in exact
    consumption order; hoisted streams use fully-resident tile tags (the
    pool can only emit WAR deps against already-emitted readers).
  - phase order tT_f, xpose, scores_f, tT_b, pT_f, scores_b, uT_f, pT_b,
    uT_b, avT_f, avT_b, gates_f, gates_b keeps the PE fed while chunks drip
    in; xpose groups are per-source-chunk so they start as chunks land.
  - the PE pstate ramps 0.65->2.4GHz over ~3us: dummy warmup matmuls burn
    the initial DMA wait so real work starts at speed.
  - exp and sigmoid live in different Act function-set tables; a dummy
    sigmoid after the last exp hides the reload off the tail.
  - PSUM->SBUF copies are split so consumers start on the first slice.
"""

import numpy as np
import ml_dtypes

import concourse.bass as bass
import concourse.mybir as mybir
import concourse.tile as tile
from concourse import bacc
from concourse.bass_utils import run_bass_kernel_spmd
from concourse.masks import make_identity

N_CORES = 8
B, T, E, H = 1024, 128, 1024, 512
BS = B // N_CORES          # 128 batch rows per core
NE = E // 128              # 8 e-chunks
F32 = mybir.dt.float32
F32R = mybir.dt.float32r
BF16 = mybir.dt.bfloat16
NPBF16 = ml_dtypes.bfloat16


DEBUG_TAPS = False    # build with an extra "dbg" output of intermediates


def build_nc(with_attn_bias=False):
    nc = bacc.Bacc("TRN2", target_bir_lowering=False, debug=False,
                   num_devices=N_CORES)

    def din(name, shape, dt=BF16):
        return nc.dram_tensor(name, shape, dt, kind="ExternalInput").ap()

    ext = {}
    for d in ("f", "b"):
        G = 3 * H if d == "f" else 2 * H
        ext[d] = {
            "A": din(f"A_{d}", [E, E]),
            "WvoT": din(f"WvoT_{d}", [E, E]),
            "WihT": din(f"WihT_{d}", [E, G]),
            "bih": din(f"bih_{d}", [1, G]),
        }
        if with_attn_bias:
            ext[d]["tb"] = din(f"tb_{d}", [1, E])
    ext["b"]["xTs"] = din("xTs_b", [128, E])
    xTs_f_ext = din("xTs_f", [128, E])
    xT_full_ext = din("xT_full", [E, B])
    out_ext = nc.dram_tensor("out", [BS, 2 * H], F32, kind="ExternalOutput").ap()
    dbg_ext = None
    if DEBUG_TAPS:
        dbg_ext = nc.dram_tensor("dbg", [BS, 8 * 1024], BF16,
                                 kind="ExternalOutput").ap()

    with tile.TileContext(nc) as tc:
        with (
            tc.tile_pool(name="sb", bufs=1) as sb_pool,
            tc.tile_pool(name="ps", bufs=1, space="PSUM") as ps_pool,
        ):
            class P:
                def __init__(self, pool, defaults):
                    self.pool, self.defaults = pool, defaults

                def tile(self, shape, dtype, name=None, tag=""):
                    bufs = self.defaults.get(tag, 1)
                    return self.pool.tile(shape, dtype, name=name, tag=tag,
                                          bufs=bufs)

            # NOTE: tags whose DMAs are hoisted before their readers are
            # emitted need full residency (bufs = NE): the tile pool can only
            # add WAR deps against readers that already exist at issue time.
            sb = P(sb_pool, {"wa_f": 8, "wa_b": 8, "wvo_f": 8, "wvo_b": 8,
                             "wih_f": 8, "wih_b": 8, "bias": 4, "act": 8,
                             "gate": 8, "stat": 4, "xts": 1, "xt": 8, "xn": 8})
            ps = P(ps_pool, {"mm": 2, "tp": 2})

            ident_f = sb_pool.tile([128, 128], F32, name="ident_f",
                                   tag="ident_f")
            make_identity(nc, ident_f)
            ident = sb_pool.tile([128, 128], BF16, name="ident", tag="ident")
            nc.vector.tensor_copy(ident[:], ident_f[:])
            ones_f = sb_pool.tile([1, 128], F32, name="ones_f", tag="ones_f")
            nc.gpsimd.memset(ones_f[:], 1.0)
            ones = sb_pool.tile([1, 128], BF16, name="ones", tag="ones")
            nc.vector.tensor_copy(ones[:], ones_f[:])

            out_f = sb_pool.tile([BS, H], F32, name="out_f", tag="out")
            out_b = sb_pool.tile([BS, H], F32, name="out_b", tag="out2")

            xTs_f = sb_pool.tile([128, E], BF16, name="xTs_f", tag="xtsf")

            dbg_sb = None
            if DEBUG_TAPS:
                dbg_sb = sb_pool.tile([BS, 8 * 1024], BF16, name="dbg_sb",
                                      tag="dbg")

            fargs = dict(nc=nc, sb=sb, ps=ps, ident=ident, ones=ones,
                         out_f=out_f, out_b=out_b, out_ext=out_ext,
                         with_attn_bias=with_attn_bias,
                         xTs_f=xTs_f, xTs_f_ext=xTs_f_ext, dbg_sb=dbg_sb)
            _emit_interleaved(ext=ext, xT_full_ext=xT_full_ext, **fargs)
            if DEBUG_TAPS:
                nc.scalar.dma_start(dbg_ext[:], dbg_sb[:])

    nc.compile()
    return nc


def _emit_interleaved(nc, sb, ps, ext, ident, ones, out_f, out_b, out_ext,
                      with_attn_bias, xTs_f, xTs_f_ext, xT_full_ext,
                      dbg_sb=None):
    """Phase-interleaved emission of both directions.

    DMA queue order:  xTs_f, A_f, xt, A_b, smalls, WvoT_f, WvoT_b, Wih_f, Wih_b
    PE order:         tT_f, xpose, scores_f, tT_b, pT_f, scores_b, uT_f, pT_b,
                      uT_b, avT_f, avT_b, gates_f, gates_b
    so the PE always has independent work while the weight stream drips in.
    """
    Sig = mybir.ActivationFunctionType.Sigmoid
    Tanh = mybir.ActivationFunctionType.Tanh

    def tap(k, src_):
        if dbg_sb is not None:
            nc.vector.tensor_copy(dbg_sb[:, k * 1024:(k + 1) * 1024], src_)

    st = {d: {"G": 3 * H if d == "f" else 2 * H} for d in ("f", "b")}
    deferred_dmas = []

    # --- DMA group: A_b[0], xTs_f, A_b[1:], xt chunks, A_f chunks, smalls --
    # The BACKWARD chain leads everywhere: its gates consume the final weight
    # stream, so its long dependency chain must start earliest. A_b[0] first
    # then xTs_f overlaps their 900ns completion-semaphore delays.
    a_chunks = {"f": [], "b": []}
    at0 = sb.tile([128, E], BF16, name="a_f_0", tag="wa_f")
    nc.sync.dma_start(at0[:], ext["f"]["A"][0:128, :])
    a_chunks["f"].append(at0)
    nc.sync.dma_start(xTs_f[:], xTs_f_ext[:])
    for ec in range(1, NE):
        at = sb.tile([128, E], BF16, name=f"a_f_{ec}", tag="wa_f")
        nc.sync.dma_start(at[:], ext["f"]["A"][ec * 128:(ec + 1) * 128, :])
        a_chunks["f"].append(at)
    xt = []
    for ec in range(NE):
        t_ = sb.tile([128, B], BF16, name=f"xt_{ec}", tag="xt")
        nc.sync.dma_start(t_[:], xT_full_ext[ec * 128:(ec + 1) * 128, :])
        xt.append(t_)
    for ec in range(NE):
        at = sb.tile([128, E], BF16, name=f"a_b_{ec}", tag="wa_b")
        nc.sync.dma_start(at[:], ext["b"]["A"][ec * 128:(ec + 1) * 128, :])
        a_chunks["b"].append(at)
    xTs_b = sb.tile([128, E], BF16, name="xTs_b", tag="xts")
    nc.sync.dma_start(xTs_b[:], ext["b"]["xTs"][:])
    bih_sb = {}
    for d in ("f", "b"):
        G = st[d]["G"]
        bih_sb[d] = sb.tile([1, G], BF16, name=f"bih_{d}", tag="bias")
        nc.sync.dma_start(bih_sb[d][:], ext[d]["bih"][:])
    tb_sb = {}
    if with_attn_bias:
        for d in ("f", "b"):
            tb_sb[d] = sb.tile([1, E], BF16, name=f"tb_{d}", tag="bias")
            nc.sync.dma_start(tb_sb[d][:], ext[d]["tb"][:])

    # --- phase helpers ------------------------------------------------------
    def do_tT(d):
        # tT[e',i] = sum_e A[e,e'] xT[e,i]; rhs is xTs_f for BOTH dirs
        # (the backward flip is folded into A_b). start=True zeroes the whole
        # 2KB PSUM bank, so only the first block of each bank asserts it.
        tT_ps = ps.tile([128, E], F32, name=f"tT_{d}", tag="mm")
        for ec in range(NE):
            for blk in range(NE):
                nc.tensor.matmul(
                    tT_ps[:, blk * 128:(blk + 1) * 128],
                    a_chunks[d][ec][:, blk * 128:(blk + 1) * 128],
                    xTs_f[:, ec * 128:(ec + 1) * 128],
                    start=(ec == 0 and blk % 4 == 0),
                    stop=(ec == NE - 1 and not with_attn_bias),
                )
        if with_attn_bias:
            for blk in range(NE):
                nc.tensor.matmul(
                    tT_ps[:, blk * 128:(blk + 1) * 128],
                    tb_sb[d][0:1, blk * 128:(blk + 1) * 128],
                    ones[0:1, :],
                    start=False, stop=True,
                )
        tT = sb.tile([128, E], BF16, name=f"tTs_{d}", tag="act")
        for p_ in range(4):
            nc.vector.tensor_copy(tT[:, p_ * 256:(p_ + 1) * 256],
                                  tT_ps[:, p_ * 256:(p_ + 1) * 256])
        if d == "f":
            tap(0, tT_ps[:])
        st[d]["tT"] = tT

    def do_xpose():
        # x natural tiles by PE transpose of the xT chunks (cheaper than a
        # second 2MB x_full DMA: HBM bytes are the scarcer resource).
        # Grouped by SOURCE chunk: xm[blk][j_lo, jc*128+e_lo] comes from
        # xt[blk] alone, so each group runs as soon as its chunk lands.
        xn = []
        for blk in range(NE):
            tp_ps = ps.tile([128, E], BF16, name=f"xn_ps_{blk}", tag="tp")
            for jc in range(NE):
                nc.tensor.transpose(tp_ps[:, jc * 128:(jc + 1) * 128],
                                    xt[blk][:, jc * 128:(jc + 1) * 128],
                                    ident[:])
            xc = sb.tile([128, E], BF16, name=f"xm_{blk}", tag="xn")
            if blk % 2 == 0:
                nc.vector.tensor_copy(xc[:], tp_ps[:])
            else:
                nc.scalar.copy(xc[:], tp_ps[:])
            xn.append(xc)
        return xn

    def do_scores(d):
        scores = ps.tile([128, B], F32, name=f"scores_{d}", tag="mm")
        tT = st[d]["tT"]
        for ec in range(NE):
            for n in range(B // 512):
                nc.tensor.matmul(
                    scores[:, n * 512:(n + 1) * 512],
                    tT[:, ec * 128:(ec + 1) * 128],
                    xt[ec][:, n * 512:(n + 1) * 512],
                    start=(ec == 0), stop=(ec == NE - 1),
                )
        if d == "f":
            tap(1, scores[:])
        # softmax: scores ~ N(0,1), exp cannot overflow -> no max-subtract
        p_un = sb.tile([128, B], BF16, name=f"p_{d}", tag="act")
        rowsum = sb.tile([128, 1], F32, name=f"rowsum_{d}", tag="stat")
        nc.scalar.activation(p_un[:], scores[:],
                             mybir.ActivationFunctionType.Exp,
                             accum_out=rowsum[:])
        rinv = sb.tile([128, 1], F32, name=f"rinv_{d}", tag="stat")
        nc.vector.reciprocal(rinv[:], rowsum[:])
        p_norm = sb.tile([128, B], BF16, name=f"pn_{d}", tag="act")
        nc.vector.tensor_scalar_mul(p_norm[:], p_un[:], rinv[:])
        st[d]["p_norm"] = p_norm

    def do_pT(d):
        p_norm = st[d]["p_norm"]
        pT_ps = ps.tile([128, B], BF16, name=f"pT_{d}", tag="tp")
        for jc in range(NE):
            nc.tensor.transpose(pT_ps[:, jc * 128:(jc + 1) * 128],
                                p_norm[:, jc * 128:(jc + 1) * 128], ident[:])
        pT = sb.tile([128, B], BF16, name=f"pTs_{d}", tag="act")
        for p_ in range(2):
            nc.vector.tensor_copy(pT[:, p_ * 512:(p_ + 1) * 512],
                                  pT_ps[:, p_ * 512:(p_ + 1) * 512])
        if d == "f":
            tap(2, p_norm[:])
            tap(3, pT_ps[:])
        st[d]["pT"] = pT

    def do_uT(d, xn):
        uT_ps = ps.tile([128, E], F32, name=f"uT_{d}", tag="mm")
        pT = st[d]["pT"]
        for jc in range(NE):
            for blk in range(NE):
                nc.tensor.matmul(
                    uT_ps[:, blk * 128:(blk + 1) * 128],
                    xn[blk][:, jc * 128:(jc + 1) * 128],
                    pT[:, jc * 128:(jc + 1) * 128],
                    start=(jc == 0 and blk % 4 == 0), stop=(jc == NE - 1),
                )
        uT = sb.tile([128, E], BF16, name=f"uTs_{d}", tag="act")
        for p_ in range(4):
            nc.scalar.copy(uT[:, p_ * 256:(p_ + 1) * 256],
                           uT_ps[:, p_ * 256:(p_ + 1) * 256])
        if d == "f":
            tap(4, uT_ps[:])
        st[d]["uT"] = uT

    def do_avT(d):
        wvo_chunks = []
        for ec in range(NE):
            wt = sb.tile([128, E], BF16, name=f"wvo_{d}_{ec}", tag=f"wvo_{d}")
            nc.sync.dma_start(wt[:], ext[d]["WvoT"][ec * 128:(ec + 1) * 128, :])
            wvo_chunks.append(wt)
        avT_ps = ps.tile([128, E], F32, name=f"avT_{d}", tag="mm")
        uT = st[d]["uT"]
        for ec in range(NE):
            for blk in range(NE):
                nc.tensor.matmul(
                    avT_ps[:, blk * 128:(blk + 1) * 128],
                    wvo_chunks[ec][:, blk * 128:(blk + 1) * 128],
                    uT[:, ec * 128:(ec + 1) * 128],
                    start=(ec == 0 and blk % 4 == 0), stop=(ec == NE - 1),
                )
        lstm_T = sb.tile([128, E], BF16, name=f"lstmT_{d}", tag="act")
        xadd = xTs_f if d == "f" else xTs_b
        nc.vector.tensor_add(lstm_T[:], avT_ps[:], xadd[:])
        if d == "f":
            tap(5, avT_ps[:])
            tap(6, lstm_T[:])
        st[d]["lstm_T"] = lstm_T

    def do_gates(d):
        G = st[d]["G"]
        gates = ps.tile([128, G], F32, name=f"gates_{d}", tag="mm")
        lstm_T = st[d]["lstm_T"]
        # bias FIRST: it opens each bank's accumulation (start=True zeroes the
        # 2KB bank), so each region completes on its last Wih chunk
        for n in range(G // 512):
            nc.tensor.matmul(
                gates[:, n * 512:(n + 1) * 512],
                ones[0:1, :],
                bih_sb[d][0:1, n * 512:(n + 1) * 512],
                start=True, stop=False,
            )
        for ec in range(NE):
            wih = sb.tile([128, G], BF16, name=f"wih_{d}_{ec}", tag=f"wih_{d}")
            nc.sync.dma_start(wih[:], ext[d]["WihT"][ec * 128:(ec + 1) * 128, :])
            for n in range(G // 512):
                nc.tensor.matmul(
                    gates[:, n * 512:(n + 1) * 512],
                    lstm_T[:, ec * 128:(ec + 1) * 128],
                    wih[:, n * 512:(n + 1) * 512],
                    start=False, stop=(ec == NE - 1),
                )
        if d == "f":
            tap(7, gates[:, 0:1024])
        si = sb.tile([128, H], F32, name=f"si_{d}", tag="gate")
        nc.scalar.activation(si[:], gates[:, 0:H], Sig)
        tg = sb.tile([128, H], F32, name=f"tg_{d}", tag="gate")
        nc.scalar.activation(tg[:], gates[:, H:2 * H], Tanh)
        if d == "f":
            # so before tc: frees the Act engine sooner for the b-direction
            so = sb.tile([128, H], F32, name=f"so_{d}", tag="gate")
            nc.scalar.activation(so[:], gates[:, 2 * H:3 * H], Sig)
            cst = sb.tile([128, H], F32, name=f"c_{d}", tag="gate")
            nc.vector.tensor_mul(cst[:], si[:], tg[:])
            tc_ = sb.tile([128, H], F32, name=f"tc_{d}", tag="gate")
            nc.scalar.activation(tc_[:], cst[:], Tanh)
            for h_ in range(2):
                sl = slice(h_ * (H // 2), (h_ + 1) * (H // 2))
                nc.vector.tensor_mul(out_f[:, sl], so[:, sl], tc_[:, sl])
                # DMA trigger deferred: an early enqueue on the in-order sync
                # queue would head-of-line block the b-direction Wih stream
                deferred_dmas.append((out_ext[:, h_ * (H // 2):
                                              (h_ + 1) * (H // 2)],
                                      out_f[:, sl]))
        else:
            for h_ in range(2):
                sl = slice(h_ * (H // 2), (h_ + 1) * (H // 2))
                nc.vector.tensor_mul(out_b[:, sl], si[:, sl], tg[:, sl])
                deferred_dmas.append((out_ext[:, H + h_ * (H // 2):
                                              H + (h_ + 1) * (H // 2)],
                                      out_b[:, sl]))

    # --- PE pstate warmup: the tensor engine ramps 0.65->2.4GHz over ~3us of
    # continuous execution; burn the DMA-wait window on dummy matmuls so the
    # real matmuls start at full clock ---
    warm = ps.tile([128, 512], F32, name="warm", tag="tp")
    for w_ in range(16):
        nc.tensor.matmul(warm[:, 0:128], ident[:], ident[:],
                         start=True, stop=True)


    # --- the schedule -------------------------------------------------------
    do_tT("f")
    xn = do_xpose()
    do_scores("f")
    do_tT("b")
    do_pT("f")
    do_scores("b")
    do_uT("f", xn)
    do_pT("b")
    do_uT("b", xn)
    do_avT("f")
    do_avT("b")
    do_gates("f")
    do_gates("b")
    for dst, src_ in deferred_dmas:
        nc.sync.dma_start(dst, src_)


_NC_CACHE = {}


def _get_nc(with_attn_bias):
    if with_attn_bias not in _NC_CACHE:
        _NC_CACHE[with_attn_bias] = build_nc(with_attn_bias)
    return _NC_CACHE[with_attn_bias]


def _bf16(a):
    return np.ascontiguousarray(a.astype(NPBF16))


def _prep_host(Wqkv, bqkv, Wo, bo, W_ih, b_ih, b_hh, flip):
    """Per-direction weight-only folds (shared across cores)."""
    Wq, Wk, Wv = Wqkv[0:E], Wqkv[E:2 * E], Wqkv[2 * E:3 * E]
    A = (Wq.T @ Wk) / 32.0
    Wvo = Wo @ Wv
    if flip:
        A = A[::-1, ::-1]
        WvoT = Wvo.T[::-1, :]          # fold input col-flip into contraction
    else:
        WvoT = Wvo.T
    blstm = b_ih + b_hh
    # fold the v/o attention biases through the lstm: af += Wo bv + bo
    att_b = Wo @ bqkv[2 * E:3 * E] + bo
    if flip:    # backward: only i and g gates are used
        W_sel = np.concatenate([W_ih[0:H], W_ih[2 * H:3 * H]], axis=0)
        b_sel = np.concatenate([blstm[0:H], blstm[2 * H:3 * H]])
    else:       # forward: i, g, o
        W_sel = np.concatenate([W_ih[0:H], W_ih[2 * H:3 * H],
                                W_ih[3 * H:4 * H]], axis=0)
        b_sel = np.concatenate([blstm[0:H], blstm[2 * H:3 * H],
                                blstm[3 * H:4 * H]])
    bih = b_sel + W_sel @ att_b
    # q-bias folded as a rank-1 row into t; k-bias is softmax-invariant
    tb = (bqkv[0:E] @ Wk) / 32.0
    if flip:
        tb = tb[::-1]
    out = {
        "A": _bf16(A),
        "WvoT": _bf16(WvoT),
        "WihT": _bf16(W_sel.T),
        "bih": _bf16(bih.reshape(1, -1)),
        "tb": _bf16(tb.reshape(1, -1)),
    }
    return out


def _pack_xts(x_rows):
    """[128, E] rows -> e-chunk-major transposed layout [128, NE*128]:
    out[p, ec*128 + i] = x_rows[i, ec*128 + p]."""
    t = x_rows.T.reshape(NE, 128, 128).transpose(1, 0, 2).reshape(128, NE * 128)
    return np.ascontiguousarray(t)


def kernel(inputs, Wqkv_f, bqkv_f, Wo_f, bo_f, W_ih_f, b_ih_f, b_hh_f,
           Wqkv_b, bqkv_b, Wo_b, bo_b, W_ih_b, b_ih_b, b_hh_b):
    inputs = np.asarray(inputs, dtype=np.float32)
    x_last = np.ascontiguousarray(inputs[:, -1, :])          # [B, E]
    xr = x_last[:, ::-1]

    shared_f = _prep_host(np.asarray(Wqkv_f), np.asarray(bqkv_f),
                          np.asarray(Wo_f), np.asarray(bo_f),
                          np.asarray(W_ih_f), np.asarray(b_ih_f),
                          np.asarray(b_hh_f), flip=False)
    shared_b = _prep_host(np.asarray(Wqkv_b), np.asarray(bqkv_b),
                          np.asarray(Wo_b), np.asarray(bo_b),
                          np.asarray(W_ih_b), np.asarray(b_ih_b),
                          np.asarray(b_hh_b), flip=True)

    with_attn_bias = bool(
        np.any(np.asarray(bqkv_f)) or np.any(np.asarray(bqkv_b)))

    xT_full = _bf16(x_last.T)                                # [E, B]

    in_maps = []
    for ci in range(N_CORES):
        rows = slice(ci * BS, (ci + 1) * BS)
        m = {"xT_full": xT_full,
             "xTs_f": _bf16(_pack_xts(x_last[rows])),
             "xTs_b": _bf16(_pack_xts(xr[rows]))}
        for d, shared in (("f", shared_f), ("b", shared_b)):
            for k in ("A", "WvoT", "WihT", "bih"):
                m[f"{k}_{d}"] = shared[k]
            if with_attn_bias:
                m[f"tb_{d}"] = shared["tb"]
        in_maps.append(m)

    nc = _get_nc(with_attn_bias)
    res = run_bass_kernel_spmd(nc, in_maps, core_ids=list(range(N_CORES)))
    out = np.concatenate([res.results[ci]["out"] for ci in range(N_CORES)],
                         axis=0)
    return out.astype(np.float32)
